# revision 22
# baseline (speedup 1.0000x reference)
"""3x3 morphological dilation (== 3x3 stride-1 max-pool) on Trainium2.

Input:  img [16, 8, 512, 512] f32 in [0, 1).
Output: out[b,c,y,x] = max over the 3x3 window of img (border padded with -2,
        which never wins since img >= 0 -- so replicate padding is equivalent).

Strategy (8 NeuronCores, pure data parallel over H), v5:
  - All device I/O and compute in fp16 (max() is exact on fp16; only input
    rounding error, rel <= 2^-11, vs the 2e-2 gate). Halves HBM traffic and
    enables the DVE 2x_1P perf mode (16-bit dtype, innermost step 1, 4B
    aligned) for every tensor_tensor max.
  - HOST de-interleaves columns: strip[.., 0:256] = even cols (E plane),
    strip[.., 256:512] = odd cols (O plane). All horizontal-window shifts
    then become dense plane ops; the two 1-elem-shifted (misaligned) copies
    are produced by the ACT engine, keeping DVE at 2x always:
      P'[i]   = max(row[2i+1], row[2i+2])        pairwise vertical
      vmE[2i] = max(row[2i], P'[i]), vmO[2i+1] = max(P'[i], row[2i+3])
      T[j]    = max(vmE[j], vmO[j])              in-pair horizontal max
      OS[j]   = vmO[j-1], ES[j] = vmE[j+1]       (ACT copies)
      outE    = max(T, OS); outO = max(T, ES)
    This hits the 2-ary-op lower bound (3 elem-ops/output) at 2 elem/cyc.
    (GpSimd copies and a broadcast-T variant both measured slower: Q7
    copies run ~4.5 cyc/vec, and stride-0 APs drop the DVE to ~2 cyc/elem.)
  - The whole 66-row strip lives in SBUF (67.6KB/partition), loaded by 5
    chunked DMAs with NO write-after-read hazards -- all loads are issued
    up-front and stream at full rate; compute never waits on a late load.
  - DMA access patterns are flattened to 2D so descriptors coalesce across
    rows instead of going out as 1KB-per-row packets.
  - Tile t's output ops are emitted after tile t+1's phase-1, giving ACT a
    full tile of slack for the shifted copies.
  - Host slices each core an overlapping strip of ALL 128 (b,c) images:
    rows [64k-1 .. 64k+64] (66 rows, edge rows replicated at the global
    top/bottom which is max-equivalent to -2 padding), and re-interleaves
    the output columns.
"""

import numpy as np

import concourse.bass as bass
import concourse.tile as tile
from concourse import bacc, mybir
from concourse.bass_utils import run_bass_kernel_spmd

N_CORES = 8
B, C, H, W = 16, 8, 512, 512
NIMG = B * C                     # 128 -> partition dim
HW_ = W // 2                     # 256 cols per plane
ROWS_PER_CORE = H // N_CORES     # 64
STRIP_ROWS = ROWS_PER_CORE + 2   # 66 (1 halo row each side)
TILE_PLAN = (4, 12, 16, 16, 16)  # output rows per tile (sums to 64)
F16 = mybir.dt.float16

_compiled = {}


def _build_nc():
    nc = bacc.Bacc(
        "TRN2",
        target_bir_lowering=False,
        debug=False,
        num_devices=N_CORES,
    )
    img = nc.dram_tensor(
        "img", [NIMG, STRIP_ROWS, W], F16, kind="ExternalInput"
    ).ap()
    out = nc.dram_tensor(
        "out", [NIMG, ROWS_PER_CORE, W], F16, kind="ExternalOutput"
    ).ap()
    img_f = img.rearrange("p r c -> p (r c)")
    out_f = out.rearrange("p r c -> p (r c)")

    max_r = max(TILE_PLAN)
    starts = []
    r0 = 0
    for R in TILE_PLAN:
        starts.append(r0)
        r0 += R

    with tile.TileContext(nc) as tc:
        with (
            tc.tile_pool(name="pwork", bufs=1) as pwork,
            tc.tile_pool(name="pout", bufs=2) as pout,
        ):
            tin_f = pwork.tile([NIMG, STRIP_ROWS * W], F16, name="tin")
            tin = tin_f.rearrange("p (r c) -> p r c", c=W)
            p = pwork.tile([NIMG, max_r // 2, W], F16)
            vmps = [
                pwork.tile([NIMG, max_r, W], F16, name=f"vmp{i}")
                for i in range(2)
            ]
            tbufs = [
                pwork.tile([NIMG, max_r, HW_], F16, name=f"tb{i}")
                for i in range(2)
            ]
            osbs = [
                pwork.tile([NIMG, max_r, W], F16, name=f"osb{i}")
                for i in range(2)
            ]
            for i in range(2):
                # border cols: OS[0] = vm[x=-1], ES[255] = vm[x=512]; -2
                # never wins. The per-tile copies don't touch these cols.
                nc.gpsimd.memset(osbs[i][:, :, 0:1], -2.0)
                nc.gpsimd.memset(osbs[i][:, :, W - 1 : W], -2.0)

            # All loads have no WAR hazards (write-once strip); issue them
            # all up-front, in consumption order, ALL on the sync ring: the
            # two HWDGE queues share the same 16 SDMA engines (round-robin
            # at packet level), so a single FIFO queue in consumption order
            # delivers the next-needed rows soonest. Stores ride the other
            # ring so they can never head-of-line-block a load.
            for ti in range(len(TILE_PLAN)):
                R, r0 = TILE_PLAN[ti], starts[ti]
                lo = 0 if ti == 0 else starts[ti] + 2
                nc.sync.dma_start(
                    tin_f[:, lo * W : (r0 + R + 2) * W],
                    img_f[:, lo * W : (r0 + R + 2) * W],
                )

            def phase1(ti):
                R, r0 = TILE_PLAN[ti], starts[ti]
                h = R // 2
                vmp, tb, osb = vmps[ti % 2], tbufs[ti % 2], osbs[ti % 2]
                # vertical 3-tap max over rows (pairwise, all 2x):
                # P'[i] = max(row[r0+2i+1], row[r0+2i+2])
                # vm[2i]   = max(row[r0+2i], P'[i])
                # vm[2i+1] = max(P'[i], row[r0+2i+3])
                nc.vector.tensor_max(
                    p[:, 0:h, :],
                    tin[:, r0 + 1 : r0 + R + 1 : 2, :],
                    tin[:, r0 + 2 : r0 + R + 2 : 2, :],
                )
                nc.vector.tensor_max(
                    vmp[:, 0:R:2, :],
                    tin[:, r0 : r0 + R : 2, :],
                    p[:, 0:h, :],
                )
                nc.vector.tensor_max(
                    vmp[:, 1:R:2, :],
                    p[:, 0:h, :],
                    tin[:, r0 + 3 : r0 + R + 2 : 2, :],
                )
                # T = within-pair horizontal max (dense, 2x)
                nc.vector.tensor_max(
                    tb[:, 0:R, :], vmp[:, 0:R, 0:HW_], vmp[:, 0:R, HW_:W]
                )
                # shifted planes on ACT: OS[j] = vmO[j-1], ES[j] = vmE[j+1]
                # -- the only misaligned accesses, off the DVE.
                nc.scalar.copy(
                    osb[:, 0:R, 1:HW_], vmp[:, 0:R, HW_ : W - 1]
                )
                nc.scalar.copy(
                    osb[:, 0:R, HW_ : W - 1], vmp[:, 0:R, 1:HW_]
                )

            def phase2(ti, split=False):
                R, r0 = TILE_PLAN[ti], starts[ti]
                tb, osb = tbufs[ti % 2], osbs[ti % 2]
                o_f = pout.tile([NIMG, max_r * W], F16, tag="o")
                o = o_f.rearrange("p (r c) -> p r c", c=W)
                chunks = [(0, R // 2), (R // 2, R)] if split else [(0, R)]
                for ci, (ra, rb) in enumerate(chunks):
                    nc.vector.tensor_max(
                        o[:, ra:rb, 0:HW_],
                        tb[:, ra:rb, :],
                        osb[:, ra:rb, 0:HW_],
                    )
                    nc.vector.tensor_max(
                        o[:, ra:rb, HW_:W],
                        tb[:, ra:rb, :],
                        osb[:, ra:rb, HW_:W],
                    )
                    # stores ride the scalar ring (loads own sync), except
                    # the very last chunk which drains on the by-then-idle
                    # sync queue, overlapping the previous chunk's store.
                    eng = nc.sync if split and ci == len(chunks) - 1 else nc.scalar
                    eng.dma_start(
                        out_f[:, (r0 + ra) * W : (r0 + rb) * W],
                        o_f[:, ra * W : rb * W],
                    )

            last = len(TILE_PLAN) - 1
            for ti in range(len(TILE_PLAN)):
                phase1(ti)
                if ti > 0:
                    phase2(ti - 1)
            phase2(last, split=True)

    nc.compile()
    return nc


def _get_nc():
    if "nc" not in _compiled:
        _compiled["nc"] = _build_nc()
    return _compiled["nc"]


def _prep(img: np.ndarray) -> list[dict[str, np.ndarray]]:
    """f32 [16,8,512,512] -> 8 de-interleaved fp16 halo strips."""
    flat = np.asarray(img, dtype=np.float32).reshape(NIMG, H, W)
    flat = flat.astype(np.float16)
    di = np.empty_like(flat)
    di[:, :, 0:HW_] = flat[:, :, 0::2]
    di[:, :, HW_:W] = flat[:, :, 1::2]
    shards = []
    for k in range(N_CORES):
        lo = k * ROWS_PER_CORE - 1
        hi = k * ROWS_PER_CORE + ROWS_PER_CORE + 1
        if lo < 0:
            strip = np.concatenate([di[:, :1], di[:, 0:hi]], axis=1)
        elif hi > H:
            strip = np.concatenate([di[:, lo:], di[:, H - 1 :]], axis=1)
        else:
            strip = di[:, lo:hi]
        shards.append(np.ascontiguousarray(strip, dtype=np.float16))
    return [{"img": s} for s in shards]


def _post(parts: list[np.ndarray]) -> np.ndarray:
    """8x fp16 [128,64,512] plane outputs -> f32 [16,8,512,512]."""
    di = np.concatenate(parts, axis=1)  # [128, 512, 512] as [E | O] planes
    full = np.empty((NIMG, H, W), dtype=np.float16)
    full[:, :, 0::2] = di[:, :, 0:HW_]
    full[:, :, 1::2] = di[:, :, HW_:W]
    return full.reshape(B, C, H, W).astype(np.float32)


def kernel(img: np.ndarray, **_unused) -> np.ndarray:
    img = np.asarray(img, dtype=np.float32)
    assert img.shape == (B, C, H, W), img.shape
    nc = _get_nc()
    in_maps = _prep(img)
    last_err = None
    for attempt in range(3):
        try:
            res = run_bass_kernel_spmd(
                nc, in_maps, core_ids=list(range(N_CORES))
            )
            break
        except Exception as e:  # transient device wedge: reset + retry
            last_err = e
            import os
            import time

            os.environ["NEURON_RT_RESET_CORES"] = "1"
            try:
                import jax

                jax.clear_caches()
                if hasattr(jax, "clear_backends"):
                    jax.clear_backends()
            except Exception:
                pass
            time.sleep(2.0 * (attempt + 1))
    else:
        raise last_err
    return _post([res.results[k]["out"] for k in range(N_CORES)])
